# revision 1
# baseline (speedup 1.0000x reference)
"""Trainium2 Bass kernel for nn_Cooord_Attn (B=2,C=64,H=W=64, dual NxN attention).

Sharding: 8 cores = 2 batches x 4 query-row-quarters.
Each core:
  - computes gated coord-conv features for its 20-row slab (16 canonical + 2 halo
    rows each side, needed because the 3x3 conv tail consumes attention output),
  - computes K/V/GK for its 16 canonical rows, AllGathers them within its
    4-core batch group to get the full 4096-key set,
  - runs both attentions (x and guide; both use x's values) for its 1280
    queries with softmax computed as exp(S - b*)/rowsum where b* is a
    per-attention upper bound on S (0.5*(max||q||^2 + max||k||^2)), which keys
    the whole softmax off key-major S^T tiles and avoids any transpose,
  - rowsum rides the AV matmul as a ones-column of V^T,
  - finishes the conv tail (c1/c2/sc) on its 16 output rows.
Host assembles the 8 [64,16,64] slices into (2,64,64,64).
"""
import sys
import numpy as np
import ml_dtypes

sys.path.insert(0, "/opt/trn_rl_repo")

import concourse.bass as bass  # noqa: E402
import concourse.tile as tile  # noqa: E402
from concourse import bacc, mybir  # noqa: E402
from concourse.bass_utils import run_bass_kernel_spmd  # noqa: E402

F32 = mybir.dt.float32
AF = mybir.ActivationFunctionType
ALU = mybir.AluOpType
AX = mybir.AxisListType

B, C, H, W = 2, 64, 64, 64
N = H * W            # 4096 pixels per image
QROWS = 20           # 16 canonical + 2 halo rows each side
QN = QROWS * W       # 1280 local queries
KROWS = 16
KN = KROWS * W       # 1024 local keys
SLABR = QROWS + 2    # conv input rows = 22
PW = W + 2           # padded width 66
NT = N // 128        # 32 key tiles
HALF = QN // 2       # 640, query half per psum pass

# AllGather buffer layout (fp32 words per rank)
OFF_K = 0
OFF_GK = 65536
OFF_VT = 131072
OFF_ST = OFF_VT + 65536          # 196608, 2 stats words
AGW = 196624                     # padded per-rank words

_CACHE = {}


def _build_program():
    nc = bacc.Bacc(None, target_bir_lowering=False, debug=False, num_devices=8)

    def din(name, shape):
        return nc.dram_tensor(name, shape, F32, kind="ExternalInput")

    # per-core tensors
    cslab = din("cslab", [66, SLABR * PW])     # x slab + coord channels, padded
    gslab = din("gslab", [66, SLABR * PW])
    BF16 = mybir.dt.bfloat16
    xfull = nc.dram_tensor("xfull", [C, N], BF16, kind="ExternalInput")
    gfull = nc.dram_tensor("gfull", [C, N], BF16, kind="ExternalInput")
    maskq = din("maskq", [1, QN])
    maskc1 = din("maskc1", [C, 18 * W])
    # weights (same on all cores)
    cw = din("cw", [66, 9 * C])                # coord conv taps, [cin, tap*cout]
    wq = din("wq", [C, C]); bq = din("bq", [C, 1])
    wk = din("wk", [C, C]); bk = din("bk", [C, 1])
    wgq = din("wgq", [C, C]); bgq = din("bgq", [C, 1])
    wgk = din("wgk", [C, C]); bgk = din("bgk", [C, 1])
    vtwb = din("vtwb", [65, C])                # [Wv^T; bv] for V^T production
    c1w = din("c1w", [C, 9 * C]); c1b = din("c1b", [C, 1])
    c2w = din("c2w", [C, 9 * C]); c2b = din("c2b", [C, 1])
    scw = din("scw", [C, C]); scb = din("scb", [C, 1])
    lwm = din("lwm", [C, 1])                   # lin_w / 4096 (mean folded)
    lbc = din("lbc", [C, 1])                   # lin_b
    hlw = din("hlw", [C, 1]); hlb = din("hlb", [C, 1])   # 0.5*lin_w, 0.5*lin_b
    gam1 = din("gam1", [1, 1])
    alpha64 = din("alpha64", [C, 1])

    out_d = nc.dram_tensor("out", [C, KN], F32, kind="ExternalOutput")

    rg = [[0, 1, 2, 3], [4, 5, 6, 7]]

    with tile.TileContext(nc) as tc:
        with (
            tc.tile_pool(name="const", bufs=1) as cp,
            tc.tile_pool(name="big", bufs=1) as bp,
            tc.tile_pool(name="small", bufs=2) as sp,
            tc.tile_pool(name="dram", bufs=1, space="DRAM") as dp,
        ):
            # ---- load constants ----
            cw_s = cp.tile([66, 9 * C], F32); nc.sync.dma_start(cw_s[:], cw[:])
            wq_s = cp.tile([C, C], F32); nc.sync.dma_start(wq_s[:], wq[:])
            wk_s = cp.tile([C, C], F32); nc.sync.dma_start(wk_s[:], wk[:])
            wgq_s = cp.tile([C, C], F32); nc.sync.dma_start(wgq_s[:], wgq[:])
            wgk_s = cp.tile([C, C], F32); nc.sync.dma_start(wgk_s[:], wgk[:])
            vtwb_s = cp.tile([65, C], F32); nc.sync.dma_start(vtwb_s[:], vtwb[:])
            c1w_s = cp.tile([C, 9 * C], F32); nc.sync.dma_start(c1w_s[:], c1w[:])
            c2w_s = cp.tile([C, 9 * C], F32); nc.sync.dma_start(c2w_s[:], c2w[:])
            scw_s = cp.tile([C, C], F32); nc.sync.dma_start(scw_s[:], scw[:])
            bq_s = cp.tile([C, 1], F32); nc.sync.dma_start(bq_s[:], bq[:])
            bk_s = cp.tile([C, 1], F32); nc.sync.dma_start(bk_s[:], bk[:])
            bgq_s = cp.tile([C, 1], F32); nc.sync.dma_start(bgq_s[:], bgq[:])
            bgk_s = cp.tile([C, 1], F32); nc.sync.dma_start(bgk_s[:], bgk[:])
            c1b_s = cp.tile([C, 1], F32); nc.sync.dma_start(c1b_s[:], c1b[:])
            c2b_s = cp.tile([C, 1], F32); nc.sync.dma_start(c2b_s[:], c2b[:])
            scb_s = cp.tile([C, 1], F32); nc.sync.dma_start(scb_s[:], scb[:])
            lwm_s = cp.tile([C, 1], F32); nc.sync.dma_start(lwm_s[:], lwm[:])
            lbc_s = cp.tile([C, 1], F32); nc.sync.dma_start(lbc_s[:], lbc[:])
            hlw_s = cp.tile([C, 1], F32); nc.sync.dma_start(hlw_s[:], hlw[:])
            hlb_s = cp.tile([C, 1], F32); nc.sync.dma_start(hlb_s[:], hlb[:])
            gam1_s = cp.tile([1, 1], F32); nc.sync.dma_start(gam1_s[:], gam1[:])
            al64_s = cp.tile([C, 1], F32); nc.sync.dma_start(al64_s[:], alpha64[:])
            maskq_s = cp.tile([1, QN], F32); nc.sync.dma_start(maskq_s[:], maskq[:])
            mc1_s = cp.tile([C, 18 * W], F32); nc.sync.dma_start(mc1_s[:], maskc1[:])
            ones64 = cp.tile([C, 1], F32); nc.vector.memset(ones64[:], 1.0)

            cs_s = bp.tile([66, SLABR * PW], F32); nc.sync.dma_start(cs_s[:], cslab[:])
            gs_s = bp.tile([66, SLABR * PW], F32); nc.sync.dma_start(gs_s[:], gslab[:])
            xf_s = bp.tile([C, N], mybir.dt.bfloat16, tag="bigA"); nc.sync.dma_start(xf_s[:], xfull[:])
            gf_s = bp.tile([C, N], mybir.dt.bfloat16, tag="bigB"); nc.sync.dma_start(gf_s[:], gfull[:])

            # ---- channel attention weights (per input) ----
            def chan_attn(full_s, name):
                pm = sp.tile([C, 1], F32, tag="ca_pm")
                nc.vector.reduce_sum(pm[:], full_s[:], axis=AX.X)
                t1 = sp.tile([C, 1], F32, tag="ca_t1")
                nc.scalar.activation(t1[:], pm[:], AF.Identity, bias=lbc_s[:, 0:1], scale=lwm_s[:, 0:1])
                t2 = sp.tile([C, 1], F32, tag="ca_t2")
                nc.vector.tensor_scalar_mul(t2[:], t1[:], 0.1)
                hh = sp.tile([C, 1], F32, tag="ca_h")
                nc.vector.tensor_max(hh[:], t1[:], t2[:])
                th = sp.tile([C, 1], F32, tag="ca_th")
                nc.scalar.activation(th[:], hh[:], AF.Tanh, bias=hlb_s[:, 0:1], scale=hlw_s[:, 0:1])
                aw = cp.tile([C, 1], F32, tag="aw_" + name)
                nc.vector.tensor_scalar(aw[:], th[:], 0.5, 0.5, op0=ALU.mult, op1=ALU.add)
                return aw

            awx = chan_attn(xf_s, "x")
            awg = chan_attn(gf_s, "g")

            # ---- coord conv -> gated features xgt/ggt [65, QN] (row 64 = ones) ----
            xgt = bp.tile([65, QN], F32)
            ggt = bp.tile([65, QN], F32)
            nc.vector.memset(xgt[64:65, :], 1.0)
            nc.vector.memset(ggt[64:65, :], 1.0)

            with tc.tile_pool(name="feps", bufs=3, space="PSUM") as fp:
                def coord_conv(slab_s, aw, dst):
                    for r0, nr in ((0, 8), (8, 8), (16, 4)):
                        ps = fp.tile([C, 512], F32, tag="fe_ps")
                        slab3 = slab_s[:].rearrange("c (r w) -> c r w", w=PW)
                        for dy in range(3):
                            for dx in range(3):
                                nc.tensor.matmul(
                                    ps[:, 0:nr * W],
                                    cw_s[:, (dy * 3 + dx) * C:(dy * 3 + dx + 1) * C],
                                    slab3[:, r0 + dy:r0 + dy + nr, dx:dx + W],
                                    start=(dy == 0 and dx == 0),
                                    stop=(dy == 2 and dx == 2),
                                )
                        nc.vector.tensor_scalar_mul(
                            dst[0:C, r0 * W:(r0 + nr) * W], ps[:, 0:nr * W], aw[:, 0:1])

                coord_conv(cs_s, awx, xgt)
                coord_conv(gs_s, awg, ggt)

                # ---- 1x1 projections ----
                qx = bp.tile([C, QN], F32)
                gqx = bp.tile([C, QN], F32)
                kx = bp.tile([C, KN], F32)
                gkx = bp.tile([C, KN], F32)

                def lin(src, w_s, b_s, dst, off, n):
                    c0 = 0
                    while c0 < n:
                        cw_ = min(512, n - c0)
                        ps = fp.tile([C, 512], F32, tag="fe_ps")
                        nc.tensor.matmul(ps[:, 0:cw_], w_s[:], src[0:C, off + c0:off + c0 + cw_],
                                         start=True, stop=True)
                        nc.vector.tensor_scalar_add(dst[:, c0:c0 + cw_], ps[:, 0:cw_], b_s[:, 0:1])
                        c0 += cw_

                lin(xgt, wq_s, bq_s, qx, 0, QN)
                lin(ggt, wgq_s, bgq_s, gqx, 0, QN)
                lin(xgt, wk_s, bk_s, kx, 2 * W, KN)
                lin(ggt, wgk_s, bgk_s, gkx, 2 * W, KN)

                # V^T local [128, 8*64]
                vt_loc = bp.tile([128, 8 * C], F32)
                for t in range(8):
                    ps = fp.tile([128, C], F32, tag="fe_ps")
                    nc.tensor.matmul(ps[:], xgt[:, 2 * W + 128 * t:2 * W + 128 * (t + 1)],
                                     vtwb_s[:], start=True, stop=True)
                    nc.vector.tensor_copy(vt_loc[:, C * t:C * (t + 1)], ps[:])

                # ---- local k^2 stats (max over keys of sum_c k^2) ----
                def sq_colmax(src, n, tagp):
                    sq = bp.tile([C, QN], F32, tag="sq_tmp")
                    nc.vector.tensor_mul(sq[:, 0:n], src[:, 0:n], src[:, 0:n])
                    parts = sp.tile([1, 4], F32, tag=tagp + "_p")
                    c0, idx = 0, 0
                    while c0 < n:
                        cw_ = min(512, n - c0)
                        ps = fp.tile([1, 512], F32, tag="fe_ps")
                        nc.tensor.matmul(ps[:, 0:cw_], ones64[:], sq[0:C, c0:c0 + cw_],
                                         start=True, stop=True)
                        nc.vector.reduce_max(parts[:, idx:idx + 1], ps[0:1, 0:cw_], axis=AX.X)
                        c0 += cw_
                        idx += 1
                    while idx < 4:
                        nc.vector.tensor_copy(parts[:, idx:idx + 1], parts[:, 0:1])
                        idx += 1
                    mx = sp.tile([1, 1], F32, tag=tagp)
                    nc.vector.reduce_max(mx[:], parts[0:1, :], axis=AX.X)
                    return mx

                k2x = sq_colmax(kx, KN, "k2x")
                k2g = sq_colmax(gkx, KN, "k2g")
                q2x = sq_colmax(qx, QN, "q2x")
                q2g = sq_colmax(gqx, QN, "q2g")

            # ---- AllGather K/GK/V^T/stats within batch group ----
            stats = sp.tile([1, 2], F32, tag="stats")
            nc.vector.tensor_copy(stats[:, 0:1], k2x[:])
            nc.vector.tensor_copy(stats[:, 1:2], k2g[:])

            agin = dp.tile([AGW], F32)
            agout = dp.tile([4, AGW], F32)
            nc.sync.dma_start(agin[OFF_K:OFF_K + C * KN].rearrange("(c m) -> c m", m=KN), kx[:])
            nc.sync.dma_start(agin[OFF_GK:OFF_GK + C * KN].rearrange("(c m) -> c m", m=KN), gkx[:])
            nc.sync.dma_start(
                agin[OFF_VT:OFF_VT + 65536].rearrange("(t p c) -> p t c", p=128, c=C),
                vt_loc[:].rearrange("p (t c) -> p t c", c=C))
            nc.sync.dma_start(agin[OFF_ST:OFF_ST + 2].rearrange("(o s) -> o s", o=1), stats[:])
            nc.gpsimd.collective_compute(
                "AllGather", ALU.bypass, ins=[agin.opt()], outs=[agout.opt()],
                replica_groups=rg)

            kf = bp.tile([C, N], F32, tag="bigA")
            gkf = bp.tile([C, N], F32, tag="bigB")
            vtf = bp.tile([128, NT * 65], F32)
            statsf = sp.tile([1, 8], F32, tag="statsf")
            nc.sync.dma_start(
                kf[:].rearrange("c (r m) -> c r m", m=KN),
                agout[:, OFF_K:OFF_K + C * KN].rearrange("r (c m) -> c r m", m=KN))
            nc.sync.dma_start(
                gkf[:].rearrange("c (r m) -> c r m", m=KN),
                agout[:, OFF_GK:OFF_GK + C * KN].rearrange("r (c m) -> c r m", m=KN))
            vtf4 = vtf[:].rearrange("p (u e) -> p u e", e=65)
            for r in range(4):
                nc.sync.dma_start(
                    vtf4[:, 8 * r:8 * r + 8, 0:C],
                    agout[r, OFF_VT:OFF_VT + 65536].rearrange("(t p c) -> p t c", p=128, c=C))
            nc.vector.memset(vtf4[:, :, C:65], 1.0)
            nc.sync.dma_start(
                statsf[:].rearrange("o (r s) -> o r s", s=2),
                agout[None, :, OFF_ST:OFF_ST + 2])

            # global key maxes and exp biases
            kmax = sp.tile([1, 2], F32, tag="kmax")
            nc.vector.reduce_max(kmax[:], statsf[0:1, :].rearrange("o (r s) -> o s r", s=2),
                                 axis=AX.X)

            def mk_bias(q2, koff):
                t = sp.tile([1, 1], F32, tag="bias_t" + str(koff))
                nc.vector.tensor_add(t[:], q2[:], kmax[:, koff:koff + 1])
                nc.vector.tensor_scalar_mul(t[:], t[:], -0.5)
                col = cp.tile([128, 1], F32, tag="bias_col" + str(koff))
                nc.gpsimd.partition_broadcast(col[:], t[0:1, :])
                return col

            bias_x = mk_bias(q2x, 0)
            bias_g = mk_bias(q2g, 1)

            # ---- attention (guide first, then x) ----
            ong = bp.tile([C, QN], F32)    # raw guide_out (masked, unscaled)
            ocx = bp.tile([C, QN], F32)    # gamma * x_out (masked)

            with (
                tc.tile_pool(name="aps_s", bufs=2, space="PSUM") as pss,
                tc.tile_pool(name="aps_o", bufs=2, space="PSUM") as pso,
                tc.tile_pool(name="atp", bufs=3) as atp,
            ):
                for (q_t, kf_t, bias_c, dst, gscale) in (
                    (gqx, gkf, bias_g, ong, None),
                    (qx, kf, bias_x, ocx, gam1_s),
                ):
                    for h in range(2):
                        o = pso.tile([65, HALF], F32, tag="o_ps")
                        for t in range(NT):
                            s = pss.tile([128, HALF], F32, tag="s_ps")
                            nc.tensor.matmul(s[:, 0:512], kf_t[:, 128 * t:128 * (t + 1)],
                                             q_t[:, HALF * h:HALF * h + 512],
                                             start=True, stop=True)
                            nc.tensor.matmul(s[:, 512:HALF], kf_t[:, 128 * t:128 * (t + 1)],
                                             q_t[:, HALF * h + 512:HALF * (h + 1)],
                                             start=True, stop=True)
                            at = atp.tile([128, HALF], F32, tag="at")
                            nc.scalar.activation(at[:], s[:], AF.Exp, bias=bias_c[:, 0:1],
                                                 scale=1.0)
                            nc.tensor.matmul(o[:, 0:512], vtf4[:, t, :], at[:, 0:512],
                                             start=(t == 0), stop=(t == NT - 1))
                            nc.tensor.matmul(o[:, 512:HALF], vtf4[:, t, :], at[:, 512:HALF],
                                             start=(t == 0), stop=(t == NT - 1))
                        rs = sp.tile([1, HALF], F32, tag="rs")
                        nc.vector.tensor_copy(rs[:], o[C:65, :])
                        rc = sp.tile([1, HALF], F32, tag="rc")
                        nc.vector.reciprocal(rc[:], rs[:])
                        nc.vector.tensor_mul(rc[:], rc[:], maskq_s[0:1, HALF * h:HALF * (h + 1)])
                        if gscale is not None:
                            nc.vector.tensor_scalar_mul(rc[:], rc[:], gscale[0:1, 0:1])
                        rb = sp.tile([C, HALF], F32, tag="rb")
                        nc.gpsimd.partition_broadcast(rb[:], rc[0:1, :])
                        nc.vector.tensor_mul(dst[:, HALF * h:HALF * (h + 1)], o[0:C, :], rb[:])

            # ---- combine + conv tail ----
            oc = bp.tile([C, QN], F32)
            talbum = bp.tile([C, QN], F32)
            nc.vector.tensor_scalar_mul(talbum[:], ong[:], al64_s[:, 0:1])
            nc.vector.tensor_add(oc[:], ocx[:], talbum[:])

            lks = bp.tile([C, QROWS * PW], F32)
            nc.vector.memset(lks[:], 0.0)
            lks3 = lks[:].rearrange("c (r w) -> c r w", w=PW)
            oc3 = oc[:].rearrange("c (r w) -> c r w", w=W)
            nc.vector.tensor_scalar_mul(talbum[:], oc[:], 0.1)
            nc.vector.tensor_max(lks3[:, :, 1:1 + W], oc3[:],
                                 talbum[:].rearrange("c (r w) -> c r w", w=W))

            c1s = bp.tile([C, QROWS * PW], F32)
            nc.vector.memset(c1s[:], 0.0)
            c1s3 = c1s[:].rearrange("c (r w) -> c r w", w=PW)
            mc13 = mc1_s[:].rearrange("c (r w) -> c r w", w=W)

            with tc.tile_pool(name="beps", bufs=3, space="PSUM") as bps:
                # c1 on slab rows [1,19)
                for r0, nr in ((1, 8), (9, 8), (17, 2)):
                    ps = bps.tile([C, 512], F32, tag="be_ps")
                    for dy in range(3):
                        for dx in range(3):
                            nc.tensor.matmul(
                                ps[:, 0:nr * W],
                                c1w_s[:, (dy * 3 + dx) * C:(dy * 3 + dx + 1) * C],
                                lks3[:, r0 + dy - 1:r0 + dy - 1 + nr, dx:dx + W],
                                start=(dy == 0 and dx == 0), stop=(dy == 2 and dx == 2))
                    tmp = sp.tile([C, 512], F32, tag="c1_tmp")
                    nc.vector.tensor_scalar_add(tmp[:, 0:nr * W], ps[:, 0:nr * W], c1b_s[:, 0:1])
                    tmp2 = sp.tile([C, 512], F32, tag="c1_tmp2")
                    nc.vector.tensor_scalar_mul(tmp2[:, 0:nr * W], tmp[:, 0:nr * W], 0.1)
                    nc.vector.tensor_max(tmp[:, 0:nr * W], tmp[:, 0:nr * W], tmp2[:, 0:nr * W])
                    nc.vector.tensor_mul(
                        c1s3[:, r0:r0 + nr, 1:1 + W],
                        tmp[:, 0:nr * W].rearrange("c (r w) -> c r w", w=W),
                        mc13[:, r0 - 1:r0 - 1 + nr, :])

                # c2 on slab rows [2,18) -> branch [C, KN]
                branch = bp.tile([C, KN], F32)
                for r0, nr in ((2, 8), (10, 8)):
                    ps = bps.tile([C, 512], F32, tag="be_ps")
                    for dy in range(3):
                        for dx in range(3):
                            nc.tensor.matmul(
                                ps[:, 0:nr * W],
                                c2w_s[:, (dy * 3 + dx) * C:(dy * 3 + dx + 1) * C],
                                c1s3[:, r0 + dy - 1:r0 + dy - 1 + nr, dx:dx + W],
                                start=(dy == 0 and dx == 0), stop=(dy == 2 and dx == 2))
                    nc.vector.tensor_scalar_add(branch[:, (r0 - 2) * W:(r0 - 2 + nr) * W],
                                                ps[:, 0:nr * W], c2b_s[:, 0:1])

                # sc 1x1 on oc rows [2,18), final = branch + sc * guide_out
                finalv = bp.tile([C, KN], F32)
                for c0 in (0, 512):
                    ps = bps.tile([C, 512], F32, tag="be_ps")
                    nc.tensor.matmul(ps[:], scw_s[:], oc[:, 2 * W + c0:2 * W + c0 + 512],
                                     start=True, stop=True)
                    tmp = sp.tile([C, 512], F32, tag="sc_tmp")
                    nc.vector.tensor_scalar_add(tmp[:], ps[:], scb_s[:, 0:1])
                    nc.vector.tensor_mul(tmp[:], tmp[:], ong[:, 2 * W + c0:2 * W + c0 + 512])
                    nc.vector.tensor_add(finalv[:, c0:c0 + 512], branch[:, c0:c0 + 512], tmp[:])

                nc.sync.dma_start(out_d[:], finalv[:])

    nc.compile()
    return nc


def _host_inputs(inputs):
    """Build the 8 per-core input maps from the full problem inputs."""
    x = np.asarray(inputs["x"], np.float32)
    guide = np.asarray(inputs["guide"], np.float32)
    lin_w = float(np.asarray(inputs["lin_w"]))
    lin_b = float(np.asarray(inputs["lin_b"]))
    coord_w = np.asarray(inputs["coord_w"], np.float32)   # (64, 66, 3, 3)
    gamma = float(np.asarray(inputs["gamma"]).reshape(-1)[0])
    alpha = float(np.asarray(inputs["alpha"]).reshape(-1)[0])

    # coordinate channels
    xx = (np.arange(W, dtype=np.float32) / (W - 1)) * 2 - 1
    yy = (np.arange(H, dtype=np.float32) / (H - 1)) * 2 - 1

    def col(v):
        return np.full((C, 1), v, np.float32)

    def taps(w):  # (O, I, 3, 3) -> [I, 9*O], tap-major
        o, i = w.shape[0], w.shape[1]
        out = np.zeros((i, 9 * o), np.float32)
        for dy in range(3):
            for dx in range(3):
                out[:, (dy * 3 + dx) * o:(dy * 3 + dx + 1) * o] = w[:, :, dy, dx].T
        return out

    wT = lambda k: np.ascontiguousarray(np.asarray(inputs[k], np.float32).T)
    bc = lambda k: np.asarray(inputs[k], np.float32).reshape(C, 1)

    vtwb = np.zeros((65, C), np.float32)
    vtwb[0:C] = wT("xv_w")
    vtwb[C] = np.asarray(inputs["xv_b"], np.float32)

    shared = dict(
        cw=taps(coord_w),
        wq=wT("xq_w"), bq=bc("xq_b"), wk=wT("xk_w"), bk=bc("xk_b"),
        wgq=wT("gq_w"), bgq=bc("gq_b"), wgk=wT("gk_w"), bgk=bc("gk_b"),
        vtwb=vtwb,
        c1w=taps(np.asarray(inputs["c1_w"], np.float32)), c1b=bc("c1_b"),
        c2w=taps(np.asarray(inputs["c2_w"], np.float32)), c2b=bc("c2_b"),
        scw=wT("sc_w"), scb=bc("sc_b"),
        lwm=col(lin_w / N), lbc=col(lin_b),
        hlw=col(0.5 * lin_w), hlb=col(0.5 * lin_b),
        gam1=np.full((1, 1), gamma, np.float32), alpha64=col(alpha),
    )

    in_maps = []
    for i in range(8):
        b, j = i // 4, i % 4
        r_lo = 16 * j - 3                      # slab image rows [r_lo, r_lo+22)
        cslab = np.zeros((66, SLABR, PW), np.float32)
        gslab = np.zeros((66, SLABR, PW), np.float32)
        for r in range(SLABR):
            ir = r_lo + r
            if 0 <= ir < H:
                cslab[0:C, r, 1:1 + W] = x[b, :, ir, :]
                gslab[0:C, r, 1:1 + W] = guide[b, :, ir, :]
                cslab[C, r, 1:1 + W] = xx
                cslab[C + 1, r, 1:1 + W] = yy[ir]
                gslab[C, r, 1:1 + W] = xx
                gslab[C + 1, r, 1:1 + W] = yy[ir]
        q_lo = 16 * j - 2
        maskq = np.zeros((1, QROWS, W), np.float32)
        for r in range(QROWS):
            if 0 <= q_lo + r < H:
                maskq[0, r] = 1.0
        maskc1 = np.zeros((1, 18, W), np.float32)
        for r in range(18):
            if 0 <= (16 * j - 1) + r < H:
                maskc1[0, r] = 1.0
        m = dict(shared)
        m.update(
            cslab=cslab.reshape(66, SLABR * PW),
            gslab=gslab.reshape(66, SLABR * PW),
            xfull=x[b].reshape(C, N).astype(ml_dtypes.bfloat16),
            gfull=guide[b].reshape(C, N).astype(ml_dtypes.bfloat16),
            maskq=maskq.reshape(1, QN),
            maskc1=np.broadcast_to(maskc1, (C, 18, W)).reshape(C, 18 * W).copy(),
        )
        in_maps.append(m)
    return in_maps


def kernel(**inputs):
    if "nc" not in _CACHE:
        _CACHE["nc"] = _build_program()
    nc = _CACHE["nc"]
    in_maps = _host_inputs(inputs)
    res = run_bass_kernel_spmd(nc, in_maps, core_ids=list(range(8)))
    out = np.zeros((B, C, H, W), np.float32)
    for i in range(8):
        b, j = i // 4, i % 4
        out[b, :, 16 * j:16 * j + 16, :] = res.results[i]["out"].reshape(C, KROWS, W)
    return out



# revision 3
# speedup vs baseline: 65.4891x; 65.4891x over previous
"""Trainium2 Bass kernel for nn_Cooord_Attn (B=2,C=64,H=W=64, dual NxN attention).

Sharding: 2 cores, one batch image per core (attention is per-sample, so the
batch axis is embarrassingly parallel). The other 6 cores idle; at this size
the wall clock is dominated by host<->device transfer over the axon tunnel
(~14 MB/s), so the design minimizes wire bytes:
  - x/guide ship once, bf16, stacked as one [128, 4096] tensor per core,
  - all weights ship as a single packed f32 vector (~0.6 MB, built per call),
  - the channel-attention scalars (a 64-element sigmoid of the per-channel
    image mean) are computed on host,
  - output returns as bf16 [64, 4096] per core,
  - the jitted executable, mesh, and donated-zero placeholders are cached
    across calls (first call pays the NEFF compile).
On device each core runs the whole pipeline for its image: padded coord-conv
slab -> gated features -> q/k/v projections -> two 4096x4096 softmax
attentions (key-major, exp biased by an upper bound on the logits so no
transpose or running max is needed; the softmax denominator rides the AV
matmul as a ones-column of V^T) -> conv tail (c1/c2/sc).
"""
import sys
import numpy as np
import ml_dtypes

sys.path.insert(0, "/opt/trn_rl_repo")

import concourse.bass as bass  # noqa: E402
import concourse.tile as tile  # noqa: E402
from concourse import bacc, mybir  # noqa: E402

F32 = mybir.dt.float32
BF16 = mybir.dt.bfloat16
AF = mybir.ActivationFunctionType
ALU = mybir.AluOpType
AX = mybir.AxisListType

B, C, H, W = 2, 64, 64, 64
N = H * W              # 4096 pixels
PW = W + 2             # padded width/height 66
NPAD = PW * PW         # 4356 padded pixels
NT = N // 128          # 32 key tiles
NCH = N // 512         # 8 column chunks of 512

# wpack layout (f32 words)
_SEGS = [
    ("cw", 66 * 9 * C), ("c1w", C * 9 * C), ("c2w", C * 9 * C),
    ("wq", C * C), ("wk", C * C), ("wgq", C * C), ("wgk", C * C),
    ("scw", C * C), ("vtwb", 65 * C),
    ("bq", C), ("bk", C), ("bgq", C), ("bgk", C),
    ("c1b", C), ("c2b", C), ("scb", C),
    ("awx", C), ("awg", C), ("gam", 1), ("alpha", C),
    ("plate", 2 * NPAD),
]
_OFF = {}
_p = 0
for _nm, _sz in _SEGS:
    _OFF[_nm] = _p
    _p += _sz
WPACK = _p

_CACHE = {}


def _build_program():
    nc = bacc.Bacc(None, target_bir_lowering=False, debug=False, num_devices=2)

    xg_d = nc.dram_tensor("xg", [2 * C, N], BF16, kind="ExternalInput")
    wp_d = nc.dram_tensor("wpack", [WPACK], F32, kind="ExternalInput")
    out_d = nc.dram_tensor("out", [C, N], BF16, kind="ExternalOutput")

    def wseg(name, p, c):
        o = _OFF[name]
        return wp_d[o:o + p * c].rearrange("(p c) -> p c", c=c)

    with tile.TileContext(nc) as tc:
        with (
            tc.tile_pool(name="const", bufs=1) as cp,
            tc.tile_pool(name="big", bufs=1) as bp,
            tc.tile_pool(name="small", bufs=2) as sp,
        ):
            # ---- load packed weights ----
            cw_s = cp.tile([66, 9 * C], F32); nc.sync.dma_start(cw_s[:], wseg("cw", 66, 9 * C))
            c1w_s = cp.tile([C, 9 * C], F32); nc.sync.dma_start(c1w_s[:], wseg("c1w", C, 9 * C))
            c2w_s = cp.tile([C, 9 * C], F32); nc.sync.dma_start(c2w_s[:], wseg("c2w", C, 9 * C))
            wq_s = cp.tile([C, C], F32); nc.sync.dma_start(wq_s[:], wseg("wq", C, C))
            wk_s = cp.tile([C, C], F32); nc.sync.dma_start(wk_s[:], wseg("wk", C, C))
            wgq_s = cp.tile([C, C], F32); nc.sync.dma_start(wgq_s[:], wseg("wgq", C, C))
            wgk_s = cp.tile([C, C], F32); nc.sync.dma_start(wgk_s[:], wseg("wgk", C, C))
            scw_s = cp.tile([C, C], F32); nc.sync.dma_start(scw_s[:], wseg("scw", C, C))
            vtwb_s = cp.tile([65, C], F32); nc.sync.dma_start(vtwb_s[:], wseg("vtwb", 65, C))
            bcol = {}
            for nm in ("bq", "bk", "bgq", "bgk", "c1b", "c2b", "scb", "awx", "awg", "alpha"):
                t = cp.tile([C, 1], F32, tag="b_" + nm)
                nc.sync.dma_start(t[:], wseg(nm, C, 1))
                bcol[nm] = t
            gam_s = cp.tile([1, 1], F32); nc.sync.dma_start(gam_s[:], wseg("gam", 1, 1))
            ones64 = cp.tile([C, 1], F32); nc.vector.memset(ones64[:], 1.0)

            # ---- inputs + padded coord slabs ----
            xg_s = bp.tile([2 * C, N], BF16, tag="xgbf")
            nc.sync.dma_start(xg_s[:], xg_d[:])

            cs_s = bp.tile([66, NPAD], F32, tag="slabA")
            gs_s = bp.tile([66, NPAD], F32, tag="slabB")
            nc.vector.memset(cs_s[0:C, :], 0.0)
            nc.vector.memset(gs_s[0:C, :], 0.0)
            nc.sync.dma_start(cs_s[C:66, :], wseg("plate", 2, NPAD))
            nc.sync.dma_start(gs_s[C:66, :], wseg("plate", 2, NPAD))
            cs3 = cs_s[:].rearrange("c (r w) -> c r w", w=PW)
            gs3 = gs_s[:].rearrange("c (r w) -> c r w", w=PW)
            xg3 = xg_s[:].rearrange("c (r w) -> c r w", w=W)
            nc.vector.tensor_copy(cs3[0:C, 1:1 + H, 1:1 + W], xg3[0:C])
            nc.vector.tensor_copy(gs3[0:C, 1:1 + H, 1:1 + W], xg3[C:2 * C])

            # ---- gated coord-conv features (row 64 = ones for bias folding) ----
            xgt = bp.tile([65, N], F32, tag="featA")
            ggt = bp.tile([65, N], F32, tag="featB")
            nc.vector.memset(xgt[64:65, :], 1.0)
            nc.vector.memset(ggt[64:65, :], 1.0)

            with tc.tile_pool(name="feps", bufs=3, space="PSUM") as fp:
                def coord_conv(slab3, aw, dst):
                    for g in range(8):
                        r0 = 8 * g
                        ps = fp.tile([C, 512], F32, tag="fe_ps")
                        for dy in range(3):
                            for dx in range(3):
                                nc.tensor.matmul(
                                    ps[:],
                                    cw_s[:, (dy * 3 + dx) * C:(dy * 3 + dx + 1) * C],
                                    slab3[:, r0 + dy:r0 + dy + 8, dx:dx + W],
                                    start=(dy == 0 and dx == 0),
                                    stop=(dy == 2 and dx == 2),
                                )
                        nc.vector.tensor_scalar_mul(
                            dst[0:C, r0 * W:(r0 + 8) * W], ps[:], aw[:, 0:1])

                coord_conv(cs3, bcol["awx"], xgt)
                coord_conv(gs3, bcol["awg"], ggt)

                # ---- 1x1 projections ----
                qx = bp.tile([C, N], F32, tag="projA")
                gqx = bp.tile([C, N], F32, tag="projB")
                kx = bp.tile([C, N], F32, tag="projC")
                gkx = bp.tile([C, N], F32, tag="projD")

                def lin(src, w_s, b_s, dst):
                    for g in range(NCH):
                        c0 = 512 * g
                        ps = fp.tile([C, 512], F32, tag="fe_ps")
                        nc.tensor.matmul(ps[:], w_s[:], src[0:C, c0:c0 + 512],
                                         start=True, stop=True)
                        nc.vector.tensor_scalar_add(dst[:, c0:c0 + 512], ps[:], b_s[:, 0:1])

                lin(xgt, wq_s, bcol["bq"], qx)
                lin(ggt, wgq_s, bcol["bgq"], gqx)
                lin(xgt, wk_s, bcol["bk"], kx)
                lin(ggt, wgk_s, bcol["bgk"], gkx)

                # V^T tiles [128 pixels, 65] (col 64 = ones for the row-sum)
                vtf = bp.tile([128, NT * 65], F32, tag="vt")
                vtf3 = vtf[:].rearrange("p (t e) -> p t e", e=65)
                nc.vector.memset(vtf[:], 1.0)
                for t in range(NT):
                    ps = fp.tile([128, C], F32, tag="fe_ps")
                    nc.tensor.matmul(ps[:], xgt[:, 128 * t:128 * (t + 1)],
                                     vtwb_s[:], start=True, stop=True)
                    nc.vector.tensor_copy(vtf3[:, t, 0:C], ps[:])

                # ---- max-norm stats -> exp biases ----
                sq = bp.tile([C, N], F32, tag="slabA")

                def sq_colmax(src, tagp):
                    nc.vector.tensor_mul(sq[:], src[0:C, :], src[0:C, :])
                    parts = sp.tile([1, NCH], F32, tag=tagp + "_p")
                    for g in range(NCH):
                        ps = fp.tile([1, 512], F32, tag="fe_ps")
                        nc.tensor.matmul(ps[:], ones64[:], sq[:, 512 * g:512 * (g + 1)],
                                         start=True, stop=True)
                        nc.vector.reduce_max(parts[:, g:g + 1], ps[0:1, :], axis=AX.X)
                    mx = sp.tile([1, 1], F32, tag=tagp)
                    nc.vector.reduce_max(mx[:], parts[0:1, :], axis=AX.X)
                    return mx

                k2x = sq_colmax(kx, "k2x")
                k2g = sq_colmax(gkx, "k2g")
                q2x = sq_colmax(qx, "q2x")
                q2g = sq_colmax(gqx, "q2g")

            def mk_bias(q2, k2, nm):
                t = sp.tile([1, 1], F32, tag="bias_t" + nm)
                nc.vector.tensor_add(t[:], q2[:], k2[:])
                nc.vector.tensor_scalar_mul(t[:], t[:], -0.5)
                col = cp.tile([128, 1], F32, tag="bias_col" + nm)
                nc.gpsimd.partition_broadcast(col[:], t[0:1, :])
                return col

            bias_x = mk_bias(q2x, k2x, "x")
            bias_g = mk_bias(q2g, k2g, "g")

            # ---- attention (guide first, then x; both use x's values) ----
            ong = bp.tile([C, N], F32, tag="featB")   # raw guide_out
            ocx = bp.tile([C, N], F32, tag="featA")   # gamma * x_out

            with (
                tc.tile_pool(name="aps_s", bufs=2, space="PSUM") as pss,
                tc.tile_pool(name="aps_o", bufs=2, space="PSUM") as pso,
                tc.tile_pool(name="atp", bufs=3) as atp,
            ):
                for (q_t, k_t, bias_c, dst, gscale) in (
                    (gqx, gkx, bias_g, ong, None),
                    (qx, kx, bias_x, ocx, gam_s),
                ):
                    for h in range(NCH):
                        o = pso.tile([65, 512], F32, tag="o_ps")
                        for t in range(NT):
                            s = pss.tile([128, 512], F32, tag="s_ps")
                            nc.tensor.matmul(s[:], k_t[:, 128 * t:128 * (t + 1)],
                                             q_t[:, 512 * h:512 * (h + 1)],
                                             start=True, stop=True)
                            at = atp.tile([128, 512], F32, tag="at")
                            nc.scalar.activation(at[:], s[:], AF.Exp,
                                                 bias=bias_c[:, 0:1], scale=1.0)
                            nc.tensor.matmul(o[:], vtf3[:, t, :], at[:],
                                             start=(t == 0), stop=(t == NT - 1))
                        rc = sp.tile([1, 512], F32, tag="rc")
                        nc.vector.reciprocal(rc[:], o[64:65, :])
                        if gscale is not None:
                            nc.vector.tensor_scalar_mul(rc[:], rc[:], gscale[0:1, 0:1])
                        rb = sp.tile([C, 512], F32, tag="rb")
                        nc.gpsimd.partition_broadcast(rb[:], rc[0:1, :])
                        nc.vector.tensor_mul(dst[:, 512 * h:512 * (h + 1)], o[0:C, :], rb[:])

            # ---- combine + conv tail ----
            oc = bp.tile([C, N], F32, tag="projA")
            tmpn = bp.tile([C, N], F32, tag="projC")
            nc.vector.tensor_scalar_mul(tmpn[:], ong[:], bcol["alpha"][:, 0:1])
            nc.vector.tensor_add(oc[:], ocx[:], tmpn[:])

            lks = bp.tile([C, NPAD], F32, tag="slabA")
            nc.vector.memset(lks[:], 0.0)
            lks3 = lks[:].rearrange("c (r w) -> c r w", w=PW)
            oc3 = oc[:].rearrange("c (r w) -> c r w", w=W)
            nc.vector.tensor_scalar_mul(tmpn[:], oc[:], 0.1)
            nc.vector.tensor_max(lks3[:, 1:1 + H, 1:1 + W], oc3[:],
                                 tmpn[:].rearrange("c (r w) -> c r w", w=W))

            c1s = bp.tile([C, NPAD], F32, tag="slabB")
            nc.vector.memset(c1s[:], 0.0)
            c1s3 = c1s[:].rearrange("c (r w) -> c r w", w=PW)

            branch = bp.tile([C, N], F32, tag="projB")
            finalv = bp.tile([C, N], F32, tag="projC")
            out_bf = bp.tile([C, N], BF16, tag="projD")

            with tc.tile_pool(name="beps", bufs=3, space="PSUM") as bps:
                def conv3(src3, w_s, g):
                    ps = bps.tile([C, 512], F32, tag="be_ps")
                    for dy in range(3):
                        for dx in range(3):
                            nc.tensor.matmul(
                                ps[:],
                                w_s[:, (dy * 3 + dx) * C:(dy * 3 + dx + 1) * C],
                                src3[:, 8 * g + dy:8 * g + dy + 8, dx:dx + W],
                                start=(dy == 0 and dx == 0), stop=(dy == 2 and dx == 2))
                    return ps

                # c1 + leaky -> padded slab
                for g in range(8):
                    ps = conv3(lks3, c1w_s, g)
                    tmp = sp.tile([C, 512], F32, tag="c1_tmp")
                    nc.vector.tensor_scalar_add(tmp[:], ps[:], bcol["c1b"][:, 0:1])
                    tmp2 = sp.tile([C, 512], F32, tag="c1_tmp2")
                    nc.vector.tensor_scalar_mul(tmp2[:], tmp[:], 0.1)
                    nc.vector.tensor_max(
                        c1s3[:, 8 * g + 1:8 * g + 9, 1:1 + W],
                        tmp[:].rearrange("c (r w) -> c r w", w=W),
                        tmp2[:].rearrange("c (r w) -> c r w", w=W))

                # c2 -> branch
                for g in range(8):
                    ps = conv3(c1s3, c2w_s, g)
                    nc.vector.tensor_scalar_add(branch[:, 512 * g:512 * (g + 1)],
                                                ps[:], bcol["c2b"][:, 0:1])

                # sc 1x1, final = branch + sc(oc) * guide_out
                for g in range(NCH):
                    c0 = 512 * g
                    ps = bps.tile([C, 512], F32, tag="be_ps")
                    nc.tensor.matmul(ps[:], scw_s[:], oc[:, c0:c0 + 512],
                                     start=True, stop=True)
                    tmp = sp.tile([C, 512], F32, tag="sc_tmp")
                    nc.vector.tensor_scalar_add(tmp[:], ps[:], bcol["scb"][:, 0:1])
                    nc.vector.tensor_mul(tmp[:], tmp[:], ong[:, c0:c0 + 512])
                    nc.vector.tensor_add(finalv[:, c0:c0 + 512], branch[:, c0:c0 + 512], tmp[:])

                nc.vector.tensor_copy(out_bf[:], finalv[:])
                nc.sync.dma_start(out_d[:], out_bf[:])

    nc.compile()
    return nc


def _coordplate():
    xx = (np.arange(W, dtype=np.float32) / (W - 1)) * 2 - 1
    yy = (np.arange(H, dtype=np.float32) / (H - 1)) * 2 - 1
    plate = np.zeros((2, PW, PW), np.float32)
    plate[0, 1:1 + H, 1:1 + W] = xx[None, :]
    plate[1, 1:1 + H, 1:1 + W] = yy[:, None]
    return plate.reshape(2 * NPAD)


def _taps(w):  # (O, I, 3, 3) -> [I, 9*O] tap-major
    o, i = w.shape[0], w.shape[1]
    out = np.empty((i, 9 * o), np.float32)
    for dy in range(3):
        for dx in range(3):
            out[:, (dy * 3 + dx) * o:(dy * 3 + dx + 1) * o] = w[:, :, dy, dx].T
    return out


def _host_inputs(inputs):
    """Build the concatenated per-core inputs: xg [2*128, N] bf16, wpack [2*WPACK] f32."""
    f = lambda k: np.asarray(inputs[k], np.float32)
    x, guide = f("x"), f("guide")
    lin_w, lin_b = float(f("lin_w")), float(f("lin_b"))
    gamma = float(f("gamma").reshape(-1)[0])
    alpha = float(f("alpha").reshape(-1)[0])

    xg = np.empty((2 * 2 * C, N), ml_dtypes.bfloat16)
    for b in range(B):
        xg[2 * C * b:2 * C * b + C] = x[b].reshape(C, N)
        xg[2 * C * b + C:2 * C * (b + 1)] = guide[b].reshape(C, N)

    # channel attention on host: sigmoid(lw*leaky(lw*mean+lb)+lb), per batch
    def aw_of(a):  # (B,C,H,W) -> (B,C)
        p = a.mean(axis=(2, 3), dtype=np.float32) * lin_w + lin_b
        hh = np.where(p > 0, p, np.float32(0.1) * p)
        t = hh * lin_w + lin_b
        return (1.0 / (1.0 + np.exp(-t))).astype(np.float32)

    awx, awg = aw_of(x), aw_of(guide)

    vtwb = np.empty((65, C), np.float32)
    vtwb[0:C] = f("xv_w").T
    vtwb[C] = f("xv_b")

    wp = np.empty(WPACK, np.float32)

    def put(nm, val):
        o = _OFF[nm]
        wp[o:o + val.size] = val.ravel()

    put("cw", _taps(f("coord_w")))
    put("c1w", _taps(f("c1_w"))); put("c2w", _taps(f("c2_w")))
    put("wq", np.ascontiguousarray(f("xq_w").T)); put("bq", f("xq_b"))
    put("wk", np.ascontiguousarray(f("xk_w").T)); put("bk", f("xk_b"))
    put("wgq", np.ascontiguousarray(f("gq_w").T)); put("bgq", f("gq_b"))
    put("wgk", np.ascontiguousarray(f("gk_w").T)); put("bgk", f("gk_b"))
    put("scw", np.ascontiguousarray(f("sc_w").T)); put("scb", f("sc_b"))
    put("vtwb", vtwb)
    put("c1b", f("c1_b")); put("c2b", f("c2_b"))
    put("gam", np.float32(gamma)); put("alpha", np.full(C, alpha, np.float32))
    put("plate", _CACHE.setdefault("plate", _coordplate()))

    wpc = np.concatenate([wp, wp])
    for b in range(B):
        wpc[b * WPACK + _OFF["awx"]:b * WPACK + _OFF["awx"] + C] = awx[b]
        wpc[b * WPACK + _OFF["awg"]:b * WPACK + _OFF["awg"] + C] = awg[b]
    return xg, wpc


def _setup():
    import jax
    from jax.sharding import Mesh, PartitionSpec, NamedSharding
    from jax.experimental.shard_map import shard_map
    import concourse.bass2jax as bass2jax

    nc = _build_program()
    bass2jax.install_neuronx_cc_hook()

    partition_name = nc.partition_id_tensor.name if nc.partition_id_tensor else None
    in_names, out_names, out_avals = [], [], []
    for alloc in nc.m.functions[0].allocations:
        if not isinstance(alloc, mybir.MemoryLocationSet):
            continue
        name = alloc.memorylocations[0].name
        if alloc.kind == "ExternalInput":
            if name != partition_name:
                in_names.append(name)
        elif alloc.kind == "ExternalOutput":
            out_names.append(name)
            out_avals.append(jax.core.ShapedArray(
                tuple(alloc.tensor_shape), mybir.dt.np(alloc.dtype)))
    n_params = len(in_names)
    n_outs = len(out_avals)
    in_names_all = list(in_names) + out_names + ([partition_name] if partition_name else [])

    def _body(*args):
        operands = list(args)
        if partition_name is not None:
            operands.append(bass2jax.partition_id_tensor())
        outs = bass2jax._bass_exec_p.bind(
            *operands,
            out_avals=tuple(out_avals), in_names=tuple(in_names_all),
            out_names=tuple(out_names), lowering_input_output_aliases=(),
            sim_require_finite=True, sim_require_nnan=True, nc=nc)
        return tuple(outs)

    devices = jax.devices()[:2]
    mesh = Mesh(np.asarray(devices), ("core",))
    sharding = NamedSharding(mesh, PartitionSpec("core"))
    donate = tuple(range(n_params, n_params + n_outs))
    sharded = jax.jit(
        shard_map(_body, mesh=mesh,
                  in_specs=(PartitionSpec("core"),) * (n_params + n_outs),
                  out_specs=(PartitionSpec("core"),) * n_outs,
                  check_rep=False),
        donate_argnums=donate, keep_unused=True)

    zero_shapes = [(2 * a.shape[0], *a.shape[1:]) for a in out_avals]
    zero_dtypes = [a.dtype for a in out_avals]
    zfn = jax.jit(
        lambda: tuple(jax.numpy.zeros(s, d) for s, d in zip(zero_shapes, zero_dtypes)),
        out_shardings=tuple(sharding for _ in out_avals))

    st = {"nc": nc, "in_names": in_names, "sharded": sharded, "zfn": zfn,
          "sharding": sharding}
    return st


def kernel(**inputs):
    import jax
    st = _CACHE.get("st")
    if st is None:
        st = _CACHE["st"] = _setup()

    xg, wpc = _host_inputs(inputs)
    by_name = {"xg": xg, "wpack": wpc}
    args = [by_name[n] for n in st["in_names"]]
    zeros = st["zfn"]()
    outs = st["sharded"](*args, *zeros)
    res = np.asarray(jax.device_get(outs[0]))  # [2*C, N] bf16
    return res.astype(np.float32).reshape(B, C, H, W)
